# revision 8
# baseline (speedup 1.0000x reference)
"""Trainium2 Bass kernel for a dense transformer block (pre-LN, causal MHA + FFN).

Full shapes: x [256, 256, 384], 6 heads x 64, FFN hidden 1536.
Sharding: data-parallel over batch across 8 NeuronCores (32 samples/core),
weights replicated. All matmuls run as float32r (full PE rate at N>=256,
~tf32 operand precision with fp32 PSUM accumulation).

walrus requires every fp32r matmul operand to be produced by a rounding op
(DVE/ACT with float32r output dtype) — plain DMA/gpsimd writes don't qualify.

Layout strategy per sample (T=256 -> 2 token tiles of 128):
  - LN1 applied via ACT (per-partition scale/bias -> float32r x_hat); gamma is
    folded into w_qkv rows; beta folded into per-output-channel bias vectors
    (computed as (beta/gamma) @ W' so only the gamma-scaled weights are kept).
  - x_hat transposed on PE (6 128x128 blocks) -> hT [C, T].
  - q,k computed TRANSPOSED (w_qkv slice as lhsT, hT as rhs) -> qkT [768, T];
    v computed natural (hT as lhsT) -> v [T, 384].
  - scoresT[tk, tq] per head = kT_h.T @ qT_h; exp on ACT (no max subtraction:
    scores are O(+-10) so fp32 exp is safe); causal mask applied
    multiplicatively post-exp; softmax denominators via ones-matmul on PE
    (replicated into each head's 64-row slot); av computed transposed
    (v_h as lhsT, expST as rhs) -> attnoutT [384, T] which feeds proj directly.
  - proj natural + residual; LN2 same trick (gamma folded into w1); FFN1
    transposed (f1T [1536, T]) with bias+ReLU fused in the ACT eviction;
    FFN2 natural + residual.
"""

from contextlib import ExitStack

import numpy as np

import concourse.bacc as bacc
import concourse.bass as bass
import concourse.mybir as mybir
import concourse.tile as tile
from concourse.bass_utils import run_bass_kernel_spmd
from concourse.masks import make_identity, make_upper_triangular

F32 = mybir.dt.float32
F32R = mybir.dt.float32r
AF = mybir.ActivationFunctionType
OP = mybir.AluOpType

B, T, C = 256, 256, 384
H, DH = 6, 64
NCORES = 8
NS = B // NCORES  # samples per core
EPS = 1e-5
KT = C // 128          # 3 contraction tiles over C
NQK = 6                # q,k channel tiles (768/128)
NF = (4 * C) // 128    # 12 FFN hidden tiles


def f(ap):
    return ap.bitcast(F32)


def build_program(ns=NS, reps=1):
    nc = bacc.Bacc("TRN2", target_bir_lowering=False, debug=False,
                   num_devices=NCORES)

    x_d = nc.dram_tensor("x", [ns, T, C], F32, kind="ExternalInput").ap()
    wqkv_d = nc.dram_tensor("w_qkv", [C, 3 * C], F32, kind="ExternalInput").ap()
    wproj_d = nc.dram_tensor("w_proj", [C, C], F32, kind="ExternalInput").ap()
    bproj_d = nc.dram_tensor("b_proj", [C], F32, kind="ExternalInput").ap()
    ln1g_d = nc.dram_tensor("ln1_g", [C], F32, kind="ExternalInput").ap()
    ln1b_d = nc.dram_tensor("ln1_b", [C], F32, kind="ExternalInput").ap()
    ln2g_d = nc.dram_tensor("ln2_g", [C], F32, kind="ExternalInput").ap()
    ln2b_d = nc.dram_tensor("ln2_b", [C], F32, kind="ExternalInput").ap()
    w1_d = nc.dram_tensor("w1", [C, 4 * C], F32, kind="ExternalInput").ap()
    b1_d = nc.dram_tensor("b1", [4 * C], F32, kind="ExternalInput").ap()
    w2_d = nc.dram_tensor("w2", [4 * C, C], F32, kind="ExternalInput").ap()
    b2_d = nc.dram_tensor("b2", [C], F32, kind="ExternalInput").ap()
    y_d = nc.dram_tensor("y", [ns, T, C], F32, kind="ExternalOutput").ap()

    xv = x_d.rearrange("s (i p) c -> s i p c", p=128)
    yv = y_d.rearrange("s (i p) c -> s i p c", p=128)

    with tile.TileContext(nc) as tc:
        _emit(nc, tc, ns, xv, yv, wqkv_d, wproj_d, bproj_d, ln1g_d, ln1b_d,
              ln2g_d, ln2b_d, w1_d, b1_d, w2_d, b2_d, reps=reps)
    nc.compile()
    return nc


def _emit(nc, tc, ns, xv, yv, wqkv_d, wproj_d, bproj_d, ln1g_d, ln1b_d,
          ln2g_d, ln2b_d, w1_d, b1_d, w2_d, b2_d, reps=1):
    ctx = ExitStack()
    const = ctx.enter_context(tc.tile_pool(name="const", bufs=1))
    setup_ctx = ExitStack()
    stg = setup_ctx.enter_context(tc.tile_pool(name="stg", bufs=1))
    ps_set = setup_ctx.enter_context(tc.tile_pool(name="ps_set", bufs=1,
                                                  space="PSUM"))

    # ------------- load raw weights (fp32 staging) -------------
    wqkv_s = stg.tile([128, KT, 3 * C], F32)
    nc.sync.dma_start(out=wqkv_s[:], in_=wqkv_d.rearrange("(k p) n -> p k n", p=128))
    wproj_s = stg.tile([128, KT, C], F32)
    nc.sync.dma_start(out=wproj_s[:], in_=wproj_d.rearrange("(k p) n -> p k n", p=128))
    w1_s = stg.tile([128, KT, 4 * C], F32)
    nc.sync.dma_start(out=w1_s[:], in_=w1_d.rearrange("(k p) n -> p k n", p=128))
    w2_s = stg.tile([128, NF, C], F32)
    nc.sync.dma_start(out=w2_s[:], in_=w2_d.rearrange("(k p) n -> p k n", p=128))

    g1 = const.tile([128, KT], F32)
    nc.sync.dma_start(out=g1[:], in_=ln1g_d.rearrange("(k p) -> p k", p=128))
    b1n = const.tile([128, KT], F32)
    nc.sync.dma_start(out=b1n[:], in_=ln1b_d.rearrange("(k p) -> p k", p=128))
    g2 = const.tile([128, KT], F32)
    nc.sync.dma_start(out=g2[:], in_=ln2g_d.rearrange("(k p) -> p k", p=128))
    b2n = const.tile([128, KT], F32)
    nc.sync.dma_start(out=b2n[:], in_=ln2b_d.rearrange("(k p) -> p k", p=128))
    b1f = const.tile([128, NF], F32)
    nc.sync.dma_start(out=b1f[:], in_=b1_d.rearrange("(k p) -> p k", p=128))

    brow = const.tile([1, C], F32)
    nc.sync.dma_start(out=brow[:], in_=bproj_d[None, :])
    b2row = const.tile([1, C], F32)
    nc.sync.dma_start(out=b2row[:], in_=b2_d[None, :])
    bproj_rep = const.tile([128, C], F32)
    nc.gpsimd.partition_broadcast(bproj_rep[:], brow[:])
    b2_rep = const.tile([128, C], F32)
    nc.gpsimd.partition_broadcast(b2_rep[:], b2row[:])

    ident_f = stg.tile([128, 128], F32)
    make_identity(nc, ident_f)
    ident = const.tile([128, 128], F32R)
    nc.vector.tensor_copy(ident[:], ident_f[:])
    ones_f = const.tile([128, 128], F32)
    nc.vector.memset(ones_f[:], 1.0)
    ones6 = const.tile([128, H, 64], F32R)
    for hh in range(H):
        nc.vector.tensor_copy(ones6[:, hh, :], ones_f[:, 0:64])
    m0 = const.tile([128, 128], F32)      # m0[i,j] = 1 if i<=j else 0
    make_upper_triangular(nc, m0, val=1.0, diag=True)
    m1 = const.tile([128, 256], F32)      # [zeros | m0] for the tk=1 block row
    nc.gpsimd.memset(m1[:, 0:128], 0.0)
    make_upper_triangular(nc, m1[:, 128:256], val=1.0, diag=True)
    eps_t = const.tile([128, 1], F32)
    nc.vector.memset(eps_t[:], EPS)

    # ------------- gamma-scaled float32r weights -------------
    wqkv = const.tile([128, KT, 3 * C], F32R)
    w1 = const.tile([128, KT, 4 * C], F32R)
    wproj = const.tile([128, KT, C], F32R)
    w2 = const.tile([128, NF, C], F32R)
    for kt in range(KT):
        nc.vector.tensor_scalar_mul(wqkv[:, kt, :], wqkv_s[:, kt, :],
                                    g1[:, kt:kt + 1])
        nc.vector.tensor_scalar_mul(w1[:, kt, :], w1_s[:, kt, :],
                                    g2[:, kt:kt + 1])
        nc.vector.tensor_copy(wproj[:, kt, :], wproj_s[:, kt, :])
    for nt in range(NF):
        nc.vector.tensor_copy(w2[:, nt, :], w2_s[:, nt, :])

    # bqkT[n,1] = sum_c W[c,n]*ln1_b[c]  (transposed-output bias for q,k).
    # One-time tiny matmuls on the RAW fp32 weights (float32r rejects N=1).
    bqkT = const.tile([128, NQK], F32)
    ps_bv = ps_set.tile([128, NQK], F32)
    for nt in range(NQK):
        for kt in range(KT):
            nc.tensor.matmul(ps_bv[:, nt:nt + 1],
                             lhsT=wqkv_s[:, kt, nt * 128:(nt + 1) * 128],
                             rhs=b1n[:, kt:kt + 1],
                             start=(kt == 0), stop=(kt == KT - 1))
    nc.vector.tensor_copy(bqkT[:], ps_bv[:])

    # b1totT[n,1] = b1[n] + sum_c w1[c,n]*ln2_b[c]
    b1tot = const.tile([128, NF], F32)
    ps_b1 = ps_set.tile([128, NF], F32)
    for nt in range(NF):
        for kt in range(KT):
            nc.tensor.matmul(ps_b1[:, nt:nt + 1],
                             lhsT=w1_s[:, kt, nt * 128:(nt + 1) * 128],
                             rhs=b2n[:, kt:kt + 1],
                             start=(kt == 0), stop=(kt == KT - 1))
    nc.vector.tensor_tensor(out=b1tot[:], in0=ps_b1[:], in1=b1f[:], op=OP.add)

    # bvec_v replicated [128, 384] via rank-broadcast lhsT
    bv_rep = const.tile([128, C], F32)
    ps_bvv = ps_set.tile([128, C], F32)
    brep_t = stg.tile([128, 128], F32)
    for kt in range(KT):
        nc.vector.tensor_scalar_mul(brep_t[:], ones_f[:], b1n[:, kt:kt + 1])
        nc.tensor.matmul(ps_bvv[:], lhsT=brep_t[:],
                         rhs=wqkv_s[:, kt, 768:1152],
                         start=(kt == 0), stop=(kt == KT - 1))
    nc.vector.tensor_copy(bv_rep[:], ps_bvv[:])

    setup_ctx.close()

    # ---------------- per-sample pools ----------------
    io = ctx.enter_context(tc.tile_pool(name="io", bufs=4))
    work = ctx.enter_context(tc.tile_pool(name="work", bufs=2))
    attn = ctx.enter_context(tc.tile_pool(name="attn", bufs=3))
    small = ctx.enter_context(tc.tile_pool(name="small", bufs=4))
    psA = ctx.enter_context(tc.tile_pool(name="psA", bufs=3, space="PSUM"))
    psB = ctx.enter_context(tc.tile_pool(name="psB", bufs=3, space="PSUM"))
    psS = ctx.enter_context(tc.tile_pool(name="psS", bufs=2, space="PSUM"))

    def layernorm(x_t):
        """x [128, C] fp32 -> x_hat [128, C] float32r (gamma/beta folded out)."""
        stats = small.tile([128, 6], F32)
        nc.vector.bn_stats(out=stats[:], in_=x_t[:])
        mv = small.tile([128, 2], F32)
        nc.vector.bn_aggr(out=mv[:], in_=stats[:])
        rstd = small.tile([128, 1], F32)
        nc.scalar.activation(out=rstd[:], in_=mv[:, 1:2], func=AF.Sqrt,
                             bias=eps_t[:], scale=1.0)
        nc.vector.reciprocal(out=rstd[:], in_=rstd[:])
        nmr = small.tile([128, 1], F32)
        nc.vector.tensor_scalar(out=nmr[:], in0=mv[:, 0:1], scalar1=rstd[:],
                                scalar2=-1.0, op0=OP.mult, op1=OP.mult)
        xh = work.tile([128, C], F32R, tag="xh")
        nc.scalar.activation(out=xh[:], in_=x_t[:], func=AF.Identity,
                             bias=nmr[:], scale=rstd[:])
        return xh

    def transpose_2tiles(xh0, xh1, tag):
        """x_hat [128,C] x2 (f32r) -> hT [128, KT, 256] f32r ([C, T] layout)."""
        hT = work.tile([128, KT, 256], F32R, tag=tag)
        for i, xh in enumerate((xh0, xh1)):
            for b in range(KT):
                pt = psB.tile([128, 128], F32, tag="mm384")
                nc.tensor.transpose(pt.bitcast(F32R)[:],
                                    xh[:, b * 128:(b + 1) * 128], ident[:])
                nc.scalar.copy(out=hT[:, b, i * 128:(i + 1) * 128], in_=pt[:])
        return hT

    rep_ctx = tc.For_i(0, reps, 1) if reps > 1 else None
    if rep_ctx is not None:
        rep_ctx.__enter__()
    for s in range(ns):
        x0 = io.tile([128, C], F32, tag="xin")
        x1 = io.tile([128, C], F32, tag="xin")
        nc.sync.dma_start(out=x0[:], in_=xv[s, 0])
        nc.sync.dma_start(out=x1[:], in_=xv[s, 1])

        # ---- LN1 + transpose ----
        xh0 = layernorm(x0)
        xh1 = layernorm(x1)
        hT = transpose_2tiles(xh0, xh1, "hT")

        # ---- qkT [128, NQK, 256] (f32r) ----
        qkT = work.tile([128, NQK, 256], F32R, tag="qkT")
        for nt in range(NQK):
            pq = psA.tile([128, 256], F32, tag="mm256")
            for kt in range(KT):
                nc.tensor.matmul(pq[:], lhsT=wqkv[:, kt, nt * 128:(nt + 1) * 128],
                                 rhs=hT[:, kt, :],
                                 start=(kt == 0), stop=(kt == KT - 1))
            nc.scalar.activation(out=qkT[:, nt, :], in_=pq[:], func=AF.Identity,
                                 bias=bqkT[:, nt:nt + 1], scale=1.0)

        # ---- v [128, 2, H, 128]: per head [v_h | ones] (f32r) ----
        v = work.tile([128, 2, H, 128], F32R, tag="v")
        for i in range(2):
            pv = psB.tile([128, C], F32, tag="mm384")
            for kt in range(KT):
                nc.tensor.matmul(pv[:], lhsT=hT[:, kt, i * 128:(i + 1) * 128],
                                 rhs=wqkv[:, kt, 768:1152],
                                 start=(kt == 0), stop=(kt == KT - 1))
            nc.vector.tensor_tensor(out=v[:, i, :, 0:64],
                                    in0=pv.rearrange("p (h d) -> p h d", h=H),
                                    in1=bv_rep.rearrange("p (h d) -> p h d", h=H),
                                    op=OP.add)
            nc.vector.tensor_copy(v[:, i, :, 64:128], ones6[:])

        # ---- attention (head pairs j; heads h = 2j + hp) ----
        # lhsT for av packs [v_h | ones] via a 2-block strided AP: out rows
        # 0:64 = attn@v, rows 64:128 = softmax denominators. All psum writes
        # land at partition base 0 (fp32r matmuls require dst base 0).
        attnoutT = work.tile([128, KT, 256], F32R, tag="aoT")
        for j in range(KT):
            for hp in range(2):
                h = 2 * j + hp
                qT = qkT[hp * 64:(hp + 1) * 64, j, :]
                kTh = qkT[hp * 64:(hp + 1) * 64, 3 + j, :]
                expS = attn.tile([128, 2, 256], F32R, tag="expS")
                for tk in range(2):
                    ps_s = psA.tile([128, 256], F32, tag="mm256")
                    nc.tensor.matmul(ps_s[:],
                                     lhsT=kTh[:, tk * 128:(tk + 1) * 128],
                                     rhs=qT[:], start=True, stop=True)
                    nc.scalar.activation(out=expS[:, tk, :], in_=ps_s[:],
                                         func=AF.Exp)
                # causal mask (multiplicative, post-exp)
                nc.vector.tensor_tensor(out=expS[:, 0, 0:128],
                                        in0=expS[:, 0, 0:128], in1=m0[:],
                                        op=OP.mult)
                nc.vector.tensor_tensor(out=expS[:, 1, :],
                                        in0=expS[:, 1, :], in1=m1[:],
                                        op=OP.mult)
                ps_hav = psS.tile([128, 256], F32, tag="psum_hav")
                for tk in range(2):
                    nc.tensor.matmul(ps_hav[:], lhsT=v[:, tk, h, :],
                                     rhs=expS[:, tk, :],
                                     start=(tk == 0), stop=(tk == 1))
                recip = small.tile([128, 256], F32, tag="recip")
                lo, hi = hp * 64, (hp + 1) * 64
                nc.vector.reciprocal(out=recip[lo:hi, :], in_=ps_hav[64:128, :])
                nc.vector.tensor_tensor(out=attnoutT[lo:hi, j, :],
                                        in0=ps_hav[0:64, :],
                                        in1=recip[lo:hi, :], op=OP.mult)

        # ---- proj + residual -> x2 ----
        x2 = [None, None]
        for i in range(2):
            pp = psB.tile([128, C], F32, tag="mm384")
            for j in range(KT):
                nc.tensor.matmul(pp[:], lhsT=attnoutT[:, j, i * 128:(i + 1) * 128],
                                 rhs=wproj[:, j, :],
                                 start=(j == 0), stop=(j == KT - 1))
            x2_t = work.tile([128, C], F32, tag="x2")
            xin = x0 if i == 0 else x1
            nc.vector.tensor_tensor(out=x2_t[:], in0=pp[:], in1=xin[:], op=OP.add)
            nc.vector.tensor_tensor(out=x2_t[:], in0=x2_t[:], in1=bproj_rep[:],
                                    op=OP.add)
            x2[i] = x2_t

        # ---- LN2 + transpose ----
        x2h0 = layernorm(x2[0])
        x2h1 = layernorm(x2[1])
        h2T = transpose_2tiles(x2h0, x2h1, "h2T")

        # ---- FFN1 -> f1T [128, NF, 256] with fused bias+ReLU (f32r) ----
        f1T = work.tile([128, NF, 256], F32R, tag="f1T")
        for nt in range(NF):
            pf = psA.tile([128, 256], F32, tag="mm256")
            for kt in range(KT):
                nc.tensor.matmul(pf[:], lhsT=w1[:, kt, nt * 128:(nt + 1) * 128],
                                 rhs=h2T[:, kt, :],
                                 start=(kt == 0), stop=(kt == KT - 1))
            nc.scalar.activation(out=f1T[:, nt, :], in_=pf[:], func=AF.Relu,
                                 bias=b1tot[:, nt:nt + 1], scale=1.0)

        # ---- FFN2 + residual -> y ----
        for i in range(2):
            pg = psB.tile([128, C], F32, tag="mm384")
            for nt in range(NF):
                nc.tensor.matmul(pg[:], lhsT=f1T[:, nt, i * 128:(i + 1) * 128],
                                 rhs=w2[:, nt, :],
                                 start=(nt == 0), stop=(nt == NF - 1))
            y_t = io.tile([128, C], F32, tag="yout")
            nc.vector.tensor_tensor(out=y_t[:], in0=pg[:], in1=x2[i][:], op=OP.add)
            nc.vector.tensor_tensor(out=y_t[:], in0=y_t[:], in1=b2_rep[:],
                                    op=OP.add)
            nc.sync.dma_start(out=yv[s, i], in_=y_t[:])

    if rep_ctx is not None:
        rep_ctx.__exit__(None, None, None)
    ctx.close()


_CACHED = None


def _get_program():
    global _CACHED
    if _CACHED is None:
        _CACHED = build_program()
    return _CACHED


def kernel(**inputs):
    x = np.asarray(inputs["x"], np.float32)
    shared = {k: np.asarray(v, np.float32) for k, v in inputs.items() if k != "x"}
    nc = _get_program()
    in_maps = []
    for c in range(NCORES):
        m = dict(shared)
        m["x"] = np.ascontiguousarray(x[c * NS:(c + 1) * NS])
        in_maps.append(m)
    res = run_bass_kernel_spmd(nc, in_maps, list(range(NCORES)))
    return np.concatenate([res.results[c]["y"] for c in range(NCORES)], axis=0)
